# revision 2
# baseline (speedup 1.0000x reference)
"""Locally-connected conv (BioConvolution) Trainium2 kernel.

Problem: Z[n,p,o] = relu(sum_{ijc} patch[n,p,i,j,c] * filt[p,i,j,c,o] + bias[o])
  X: (32,128,128,32) f32, filters: (1024,4,4,32,32) f32, bias: (32,)
  out: (32,32,32,32) f32.   FH=FW=4 non-overlapping patches, P=1024.

Sharding: patch-parallel over P across 8 cores. Core k owns patches
[128k,128k+128) == image rows [16k,16k+16). Each core touches only its own
X rows and filters: 16.8 MB in + 0.5 MB out per core — the true memory
roofline (no operand is reused across cores).

Host-side marshaling (part of sharding): the contraction axis must sit on
SBUF partitions for the PE, so X is pre-arranged per-core into
  xt[r, p, q, b] = X[b, 16k+4*pr+q, 4*pc+j, c]   (r = j*32+c, p = pr*32+pc)
and filters into the matching
  ft[r, p, q, o] = filters[128k+p, q, j, c, o].
Both are r-major so every HBM->SBUF DMA moves long contiguous runs per
partition (multi-KB descriptors at line rate).

Device kernel (identical SPMD program on 8 cores):
  for each 4-patch group: 16 fp32 matmuls (K=128, M=32 fout, N=32 batch)
  accumulate into one PSUM [128,32] tile (col-tiled: patch s -> partitions
  32s..32s+32), ScalarE applies bias+ReLU into an SBUF staging buffer,
  one contiguous 512 KB output DMA at the end.
"""

import numpy as np

N, H, W, C = 32, 128, 128, 32
FH = FW = 4
FOUT = 32
NCORES = 8
PL = 128          # patches per core
NQ = 4            # K-chunks per patch (512 / 128)
KR = 128          # contraction rows per chunk (SBUF partitions)
NG = PL // 4      # 4-patch groups per core

_CACHE = {}


def _build_module(chunk_patches=8):
    from concourse import bacc, tile, mybir

    nc = bacc.Bacc("TRN2", target_bir_lowering=False, debug=False)
    dt = mybir.dt.float32
    xt = nc.dram_tensor("xt", [KR, PL, NQ, N], dt, kind="ExternalInput").ap()
    ft = nc.dram_tensor("ft", [KR, PL, NQ, FOUT], dt, kind="ExternalInput").ap()
    bt = nc.dram_tensor("bt", [KR, 1], dt, kind="ExternalInput").ap()
    out = nc.dram_tensor("out", [KR, NG, N], dt, kind="ExternalOutput").ap()

    PC = chunk_patches
    NCHUNK = PL // PC
    relu = mybir.ActivationFunctionType.Relu

    with tile.TileContext(nc) as tc:
        with (
            tc.tile_pool(name="xpool", bufs=3) as xpool,
            tc.tile_pool(name="fpool", bufs=3) as fpool,
            tc.tile_pool(name="psum", bufs=8, space="PSUM") as psum,
            tc.tile_pool(name="misc", bufs=1) as misc,
        ):
            bias_t = misc.tile([KR, 1], dt)
            nc.sync.dma_start(bias_t[:], bt[:])
            staging = misc.tile([KR, NG, N], dt)

            for ch in range(NCHUNK):
                xtile = xpool.tile([KR, PC, NQ, N], dt, tag="xt")
                ftile = fpool.tile([KR, PC, NQ, FOUT], dt, tag="ft")
                sl = slice(ch * PC, (ch + 1) * PC)
                nc.sync.dma_start(xtile[:], xt[:, sl, :, :])
                nc.sync.dma_start(ftile[:], ft[:, sl, :, :])
                for g in range(PC // 4):
                    gg = ch * (PC // 4) + g
                    ptile = psum.tile([KR, N], dt, tag="ps")
                    for s in range(4):
                        p = g * 4 + s
                        for q in range(NQ):
                            nc.tensor.matmul(
                                ptile[32 * s : 32 * s + 32, :],
                                ftile[:, p, q, :],   # lhsT [128, 32(o)]
                                xtile[:, p, q, :],   # rhs  [128, 32(b)]
                                start=(q == 0),
                                stop=(q == NQ - 1),
                                tile_position=(0, 32 * s),
                            )
                    nc.scalar.activation(
                        staging[:, gg, :], ptile[:], relu, bias=bias_t[:]
                    )
            nc.sync.dma_start(out[:], staging[:])
    nc.compile()
    return nc


def _get_module():
    if "nc" not in _CACHE:
        _CACHE["nc"] = _build_module()
    return _CACHE["nc"]


def _marshal(X, filters, bias):
    """Shard + lay out full inputs into per-core device arrays."""
    X = np.ascontiguousarray(np.asarray(X, dtype=np.float32))
    filters = np.ascontiguousarray(np.asarray(filters, dtype=np.float32))
    bias = np.asarray(bias, dtype=np.float32)

    # X: (b, core, pr, i, pc, j, c) -> (core, j, c, pr, pc, i, b)
    xv = X.reshape(N, NCORES, 4, FH, 32, FW, C)
    xt = np.ascontiguousarray(xv.transpose(1, 5, 6, 2, 4, 3, 0)).reshape(
        NCORES, KR, PL, NQ, N
    )
    # filters: (core, p, i, j, c, o) -> (core, j, c, p, i, o)
    fv = filters.reshape(NCORES, PL, FH, FW, C, FOUT)
    ft = np.ascontiguousarray(fv.transpose(0, 3, 4, 1, 2, 5)).reshape(
        NCORES, KR, PL, NQ, FOUT
    )
    bt = np.ascontiguousarray(np.tile(bias, 4).reshape(KR, 1))
    return xt, ft, bt


def _assemble(outs):
    """Per-core out [128=(s,o), NG, N] -> full (N, 32, 32, FOUT)."""
    z = np.stack(outs)                                  # (core, (s,o), g, b)
    z = z.reshape(NCORES, 4, FOUT, NG, N)               # (core, s, o, g, b)
    z = z.transpose(4, 0, 3, 1, 2)                      # (b, core, g, s, o)
    z = z.reshape(N, NCORES, PL, FOUT)                  # p_loc = 4*g + s
    z = z.reshape(N, NCORES * 4, 32, FOUT)              # (b, pr_glob, pc, o)
    return np.ascontiguousarray(z)


LAST_RESULT = None


def kernel(X, filters, bias):
    global LAST_RESULT
    from concourse.bass_utils import run_bass_kernel_spmd

    nc = _get_module()
    xt, ft, bt = _marshal(X, filters, bias)
    in_maps = [
        {"xt": xt[k], "ft": ft[k], "bt": bt} for k in range(NCORES)
    ]
    res = run_bass_kernel_spmd(nc, in_maps, core_ids=list(range(NCORES)))
    LAST_RESULT = res
    return _assemble([res.results[k]["out"] for k in range(NCORES)])
